# revision 7
# baseline (speedup 1.0000x reference)
"""Multi-head attention (N=4, S=T=2048, E=1024, H=16) on 8 trn2 NeuronCores.

Sharding: 8 cores = 4 batches x 2 head-groups (Megatron-style over heads).
Each core computes 8 heads of one batch and a partial output projection;
the host sums the two partials per batch and adds the output bias.

v3: K/Q-chunk0 projections first, then V; attention starts as soon as
K+Q0 land. Later Q chunks and the deferred out-projections are emitted
as fine-grained "filler" units pumped between attention key-tiles so
the PE never bursts away from score production (which starves the
Scalar engine) and never idles long enough to re-throttle HAM.
"""
import os
import sys

for _p in ("/opt/trn_rl_repo", "/root/.axon_site/_ro/trn_rl_repo"):
    if os.path.isdir(_p) and _p not in sys.path:
        sys.path.append(_p)

import numpy as np
import ml_dtypes

import concourse.bass as bass
import concourse.mybir as mybir
import concourse.tile as tile
from concourse import bacc
from concourse.bass_utils import run_bass_kernel_spmd

F32 = mybir.dt.float32
BF16 = mybir.dt.bfloat16
AF = mybir.ActivationFunctionType

E_FULL = 1024
H_FULL = 16
HD = 64
N_FULL, S_FULL, T_FULL = 4, 2048, 2048
N_CORES = 8

# Set by the test harness to collect a profile.
TRACE = False
TRACE_KW = {}
LAST_RESULT = [None]


def _build_nc(E, S, T, NH, CH=512, PUMP_EVERY=4):
    """Per-core kernel: NH heads (dim 64), S queries, T keys, model dim E."""
    DG = NH * HD
    EB = E // 128
    DB = DG // 128
    TB = T // 128
    VW = HD + 1
    NCH = S // CH
    SP = CH // 128
    assert DG % 128 == 0 and CH % 512 == 0 and S % CH == 0

    nc = bacc.Bacc(None)
    xqT = nc.dram_tensor("xqT", [E, S], BF16, kind="ExternalInput")
    xkT = nc.dram_tensor("xkT", [E, T], BF16, kind="ExternalInput")
    xvT = nc.dram_tensor("xvT", [E, T], BF16, kind="ExternalInput")
    wqT = nc.dram_tensor("wqT", [E, DG], BF16, kind="ExternalInput")
    wkT = nc.dram_tensor("wkT", [E, DG], BF16, kind="ExternalInput")
    wvT = nc.dram_tensor("wvT", [E, DG], BF16, kind="ExternalInput")
    wpT = nc.dram_tensor("wpT", [DG, E], BF16, kind="ExternalInput")
    bq = nc.dram_tensor("bq", [DG], F32, kind="ExternalInput")
    bk = nc.dram_tensor("bk", [DG], F32, kind="ExternalInput")
    bv = nc.dram_tensor("bv", [DG], F32, kind="ExternalInput")
    out = nc.dram_tensor("out", [S, E], F32, kind="ExternalOutput")

    with tile.TileContext(nc) as tc:
        with (
            tc.tile_pool(name="const", bufs=1) as cpool,
            tc.tile_pool(name="persist", bufs=1) as ppool,
            tc.tile_pool(name="wpool", bufs=1) as wpool,
            tc.tile_pool(name="spool", bufs=3) as spool,
            tc.tile_pool(name="epool", bufs=10) as epool,
            tc.tile_pool(name="npool", bufs=2) as npool,
            tc.tile_pool(name="n1pool", bufs=2) as n1pool,
            tc.tile_pool(name="ypool", bufs=2) as ypool,
            tc.tile_pool(name="opool", bufs=2) as opool,
            tc.tile_pool(name="stp", bufs=2, space="PSUM") as stp,
            tc.tile_pool(name="ytp", bufs=2, space="PSUM") as ytp,
            tc.tile_pool(name="opp", bufs=2, space="PSUM") as opp,
        ):
            # weights for K first so the K projection starts immediately
            wk_sb = wpool.tile([128, EB, DG], BF16, tag="wk", name="wk")
            wkr = wkT.rearrange("(eb p) d -> p eb d", p=128)
            nc.sync.dma_start(wk_sb[:, 0 : EB // 2, :], wkr[:, 0 : EB // 2, :])
            nc.gpsimd.dma_start(wk_sb[:, EB // 2 : EB, :], wkr[:, EB // 2 : EB, :])
            bk_sb = cpool.tile([128, DB], F32, tag="bk")
            nc.sync.dma_start(bk_sb[:], bk.rearrange("(db p) -> p db", p=128))

            qt_sb = ppool.tile([128, DB, S], BF16, tag="qt")
            kt_sb = ppool.tile([128, DB, T], BF16, tag="kt")
            v_sb = ppool.tile([128, TB, NH * VW], BF16, tag="v")

            def proj_db(xs, w_sb, b_sb, o_sb, pc, db):
                ps = opp.tile([128, 512], F32, tag="op", name="psp")
                for kb in range(EB):
                    nc.tensor.matmul(
                        ps[:],
                        w_sb[:, kb, db * 128 : (db + 1) * 128],
                        xs[:, kb, :],
                        start=(kb == 0),
                        stop=(kb == EB - 1),
                    )
                nc.vector.tensor_scalar_add(
                    o_sb[:, db, pc * 512 : (pc + 1) * 512],
                    ps[:],
                    b_sb[:, db : db + 1],
                )

            def x_chunk(src, pc, tag, split=False):
                xs = spool.tile([128, EB, 512], BF16, tag=tag, name=tag)
                r = src.rearrange("(eb p) s -> p eb s", p=128)[
                    :, :, pc * 512 : (pc + 1) * 512
                ]
                if split:
                    # two DMAs on separate queues so the first kb matmuls
                    # start sooner
                    nc.sync.dma_start(xs[:, 0 : EB // 2, :], r[:, 0 : EB // 2, :])
                    nc.gpsimd.dma_start(xs[:, EB // 2 : EB, :], r[:, EB // 2 : EB, :])
                else:
                    nc.sync.dma_start(xs[:], r)
                return xs

            def proj_chunk(src, w_sb, b_sb, o_sb, pc, tag="xs", split=False,
                           dbs=None):
                xs = x_chunk(src, pc, tag, split=split)
                for db in dbs if dbs is not None else range(DB):
                    proj_db(xs, w_sb, b_sb, o_sb, pc, db)
                return xs

            # K chunk 0 (split x DMA for a faster first matmul)
            proj_chunk(xkT, wk_sb, bk_sb, kt_sb, 0, split=True)

            wq_sb = wpool.tile([128, EB, DG], BF16, tag="wq", name="wq")
            nc.sync.dma_start(wq_sb[:], wqT.rearrange("(eb p) d -> p eb d", p=128))
            bq_sb = cpool.tile([128, DB], F32, tag="bq")
            nc.sync.dma_start(bq_sb[:], bq.rearrange("(db p) -> p db", p=128))
            # Q chunk 0, head-pair 0 only (unblocks attention chunk 0);
            # the other head-pairs are woven into the first V group below.
            xq0 = proj_chunk(xqT, wq_sb, bq_sb, qt_sb, 0, tag="xq", split=True,
                             dbs=[0])

            wv_sb = wpool.tile([128, EB, DG], BF16, tag="wv", name="wv")
            nc.sync.dma_start(wv_sb[:], wvT.rearrange("(eb p) d -> p eb d", p=128))
            bv_sb = cpool.tile([HD, NH], F32, tag="bv")
            nc.sync.dma_start(bv_sb[:], bv.rearrange("(h p) -> p h", p=HD))
            wp_sb = cpool.tile([128, DB, E], BF16, tag="wp")
            nc.sync.dma_start(wp_sb[:], wpT.rearrange("(db p) e -> p db e", p=128))

            filler = []

            def pump():
                if filler:
                    filler.pop(0)()

            def v_tile(xs, tb):
                t4 = tb % 4
                ps = opp.tile([128, DG], F32, tag="op", name="psv")
                for kb in range(EB):
                    nc.tensor.matmul(
                        ps[:],
                        xs[:, kb, t4 * 128 : (t4 + 1) * 128],
                        wv_sb[:, kb, :],
                        start=(kb == 0),
                        stop=(kb == EB - 1),
                    )
                nc.vector.tensor_copy(
                    v_sb[:, tb, :].rearrange("p (h w) -> p h w", w=VW)[:, :, 0:HD],
                    ps[:].rearrange("p (h w) -> p h w", w=HD),
                )
                nc.vector.memset(
                    v_sb[:, tb, :].rearrange("p (h w) -> p h w", w=VW)[:, :, HD:VW],
                    1.0,
                )

            def normalize(ye, hpair, yt_sb):
                for hi, h in ((0, hpair[0]), (1, hpair[1])):
                    db_, rh = h // 2, (h % 2) * 64
                    sp = n1pool.tile([128, SP], F32, tag="sp", name="sp")
                    nc.gpsimd.dma_start(sp[:], ye[hi][64:65, :])
                    nc.vector.reciprocal(sp[:], sp[:])
                    rs = n1pool.tile([1, CH], F32, tag="rs", name="rs")
                    nc.gpsimd.dma_start(rs[:], sp[:])
                    rbc = n1pool.tile([64, CH], F32, tag="rbc", name="rbc")
                    nc.gpsimd.partition_broadcast(rbc[:], rs[:])
                    if rh == 0:
                        dst = yt_sb[0:64, db_, :]
                        nc.vector.tensor_tensor(
                            dst, ye[hi][0:64, :], rbc[:], mybir.AluOpType.mult
                        )
                        nc.vector.tensor_scalar_add(dst, dst, bv_sb[:, h : h + 1])
                    else:
                        tmp = n1pool.tile([64, CH], BF16, tag="tmp", name="tmp")
                        nc.vector.tensor_tensor(
                            tmp[:], ye[hi][0:64, :], rbc[:], mybir.AluOpType.mult
                        )
                        nc.vector.tensor_scalar_add(
                            tmp[:], tmp[:], bv_sb[:, h : h + 1]
                        )
                        nc.gpsimd.dma_start(yt_sb[64:128, db_, :], tmp[:])

            def outproj_unit(yt_sb, s0, sb, jc):
                def emit():
                    op = opp.tile([128, 512], F32, tag="op", name="op")
                    for ib in range(DB):
                        nc.tensor.matmul(
                            op[:],
                            yt_sb[:, ib, sb * 128 : (sb + 1) * 128],
                            wp_sb[:, ib, jc * 512 : (jc + 1) * 512],
                            start=(ib == 0),
                            stop=(ib == DB - 1),
                        )
                    ob = opool.tile([128, 512], F32, tag="ob")
                    nc.vector.tensor_copy(ob[:], op[:])
                    nc.sync.dma_start(
                        out[
                            s0 + sb * 128 : s0 + (sb + 1) * 128,
                            jc * 512 : (jc + 1) * 512,
                        ],
                        ob[:],
                    )

                return emit

            def qproj_unit(pc, db):
                def emit():
                    if db == 0:
                        qproj_unit.xs = x_chunk(xqT, pc, "xq")
                    proj_db(qproj_unit.xs, wq_sb, bq_sb, qt_sb, pc, db)

                return emit

            def attn_tb(hp, s0, yt0, yt1, tb):
                h0, h1 = 2 * hp, 2 * hp + 1
                st = stp.tile([128, 2 * CH], F32, tag="st", name="st")
                nc.tensor.matmul(
                    st[:, 0:CH],
                    kt_sb[0:64, hp, tb * 128 : (tb + 1) * 128],
                    qt_sb[0:64, hp, s0 : s0 + CH],
                    start=True,
                    stop=True,
                    tile_position=(0, 0),
                )
                nc.tensor.matmul(
                    st[:, CH : 2 * CH],
                    kt_sb[64:128, hp, tb * 128 : (tb + 1) * 128],
                    qt_sb[64:128, hp, s0 : s0 + CH],
                    start=True,
                    stop=True,
                    tile_position=(64, 0),
                )
                e_tb = epool.tile([128, 2 * CH], BF16, tag="e")
                nc.scalar.activation(e_tb[:], st[:], AF.Exp, scale=0.125)
                nc.tensor.matmul(
                    yt0[:],
                    v_sb[:, tb, h0 * VW : (h0 + 1) * VW],
                    e_tb[:, 0:CH],
                    start=(tb == 0),
                    stop=(tb == TB - 1),
                )
                nc.tensor.matmul(
                    yt1[:],
                    v_sb[:, tb, h1 * VW : (h1 + 1) * VW],
                    e_tb[:, CH : 2 * CH],
                    start=(tb == 0),
                    stop=(tb == TB - 1),
                )

            def attn_hp_tail(hp, yt_sb, yt0, yt1):
                h0, h1 = 2 * hp, 2 * hp + 1
                ye = [
                    npool.tile([65, CH], F32, tag="ye0", name="ye0"),
                    npool.tile([65, CH], F32, tag="ye1", name="ye1"),
                ]
                nc.vector.tensor_copy(ye[0][:], yt0[:])
                nc.vector.tensor_copy(ye[1][:], yt1[:])
                normalize(ye, (h0, h1), yt_sb)

            # ---- chunk 0, head-pair 0: interleaved with K chunks 1-3 and
            # V production so the exp stream starts as early as possible.
            yt_c0 = ypool.tile([128, DB, CH], BF16, tag="yt", name="yt_sb")
            yt0 = ytp.tile([65, CH], F32, tag="ytp", name="yt0")
            yt1 = ytp.tile([65, CH], F32, tag="ytp", name="yt1")
            for pcv in range(T // 512):
                xs = x_chunk(xvT, pcv, "xs", split=True)
                for t4 in range(4):
                    tb = pcv * 4 + t4
                    v_tile(xs, tb)
                    attn_tb(0, 0, yt0, yt1, tb)
                    if pcv == 0 and t4 < 3:
                        proj_db(xq0, wq_sb, bq_sb, qt_sb, 0, t4 + 1)
                if pcv + 1 < T // 512:
                    proj_chunk(xkT, wk_sb, bk_sb, kt_sb, pcv + 1)
            attn_hp_tail(0, yt_c0, yt0, yt1)

            # ---- remaining head-pairs / chunks, filler-pumped ----
            for c in range(NCH):
                s0 = c * CH
                if c == 0:
                    yt_sb = yt_c0
                else:
                    yt_sb = ypool.tile([128, DB, CH], BF16, tag="yt", name="yt_sb")
                for hp in range(DB):
                    if c == 0 and hp == 0:
                        continue
                    yt0 = ytp.tile([65, CH], F32, tag="ytp", name="yt0")
                    yt1 = ytp.tile([65, CH], F32, tag="ytp", name="yt1")
                    for tb in range(TB):
                        attn_tb(hp, s0, yt0, yt1, tb)
                        if tb % 5 == 2:
                            pump()
                    attn_hp_tail(hp, yt_sb, yt0, yt1)
                    if hp == 1 and c + 1 < NCH:
                        for pq in range(CH // 512):
                            pc = (c + 1) * (CH // 512) + pq
                            for db in range(DB):
                                filler.append(qproj_unit(pc, db))
                for sb in range(CH // 128):
                    for jc in range(E // 512):
                        filler.append(outproj_unit(yt_sb, s0, sb, jc))
            while filler:
                pump()

    nc.compile()
    return nc


_NC_CACHE = {}


def _get_nc(key, builder):
    if key not in _NC_CACHE:
        _NC_CACHE[key] = builder()
    return _NC_CACHE[key]


def kernel(query, key, value, Wq, bq, Wk, bk, Wv, bv, Wp, bp):
    query = np.asarray(query, np.float32)
    key = np.asarray(key, np.float32)
    value = np.asarray(value, np.float32)
    Wq, bq = np.asarray(Wq, np.float32), np.asarray(bq, np.float32)
    Wk, bk = np.asarray(Wk, np.float32), np.asarray(bk, np.float32)
    Wv, bv = np.asarray(Wv, np.float32), np.asarray(bv, np.float32)
    Wp, bp = np.asarray(Wp, np.float32), np.asarray(bp, np.float32)

    n, s, e = query.shape
    t = value.shape[1]
    assert (n, s, t, e) == (N_FULL, S_FULL, T_FULL, E_FULL)

    nc = _get_nc(
        "full",
        lambda: _build_nc(E_FULL, S_FULL, T_FULL, H_FULL // 2),
    )

    DG = (H_FULL // 2) * HD
    bf = ml_dtypes.bfloat16
    in_maps = []
    for c in range(N_CORES):
        b, g = c // 2, c % 2
        gs = slice(g * DG, (g + 1) * DG)
        in_maps.append(
            {
                "xqT": query[b].T.astype(bf),
                "xkT": key[b].T.astype(bf),
                "xvT": value[b].T.astype(bf),
                "wqT": Wq[gs, :].T.astype(bf),
                "wkT": Wk[gs, :].T.astype(bf),
                "wvT": Wv[gs, :].T.astype(bf),
                "wpT": Wp[:, gs].T.astype(bf),
                "bq": np.ascontiguousarray(bq[gs]),
                "bk": np.ascontiguousarray(bk[gs]),
                "bv": np.ascontiguousarray(bv[gs]),
            }
        )

    res = run_bass_kernel_spmd(
        nc, in_maps, list(range(N_CORES)), trace=TRACE, **TRACE_KW
    )
    LAST_RESULT[0] = res

    outp = np.empty((n, s, e), np.float32)
    for b in range(n):
        outp[b] = res.results[2 * b]["out"] + res.results[2 * b + 1]["out"] + bp
    return outp


# revision 8
# speedup vs baseline: 1.0267x; 1.0267x over previous
"""Multi-head attention (N=4, S=T=2048, E=1024, H=16) on 8 trn2 NeuronCores.

Sharding: 8 cores = 4 batches x 2 head-groups (Megatron-style over heads).
Each core computes 8 heads of one batch and a partial output projection;
the host sums the two partials per batch and adds the output bias.

v3: K/Q-chunk0 projections first, then V; attention starts as soon as
K+Q0 land. Later Q chunks and the deferred out-projections are emitted
as fine-grained "filler" units pumped between attention key-tiles so
the PE never bursts away from score production (which starves the
Scalar engine) and never idles long enough to re-throttle HAM.
"""
import os
import sys

for _p in ("/opt/trn_rl_repo", "/root/.axon_site/_ro/trn_rl_repo"):
    if os.path.isdir(_p) and _p not in sys.path:
        sys.path.append(_p)

import numpy as np
import ml_dtypes

import concourse.bass as bass
import concourse.mybir as mybir
import concourse.tile as tile
from concourse import bacc
from concourse.bass_utils import run_bass_kernel_spmd

F32 = mybir.dt.float32
BF16 = mybir.dt.bfloat16
AF = mybir.ActivationFunctionType

E_FULL = 1024
H_FULL = 16
HD = 64
N_FULL, S_FULL, T_FULL = 4, 2048, 2048
N_CORES = 8

# Set by the test harness to collect a profile.
TRACE = False
TRACE_KW = {}
LAST_RESULT = [None]


def _build_nc(E, S, T, NH, CH=512, PUMP_EVERY=4):
    """Per-core kernel: NH heads (dim 64), S queries, T keys, model dim E."""
    DG = NH * HD
    EB = E // 128
    DB = DG // 128
    TB = T // 128
    VW = HD + 1
    NCH = S // CH
    SP = CH // 128
    assert DG % 128 == 0 and CH % 512 == 0 and S % CH == 0

    nc = bacc.Bacc(None)
    xqT = nc.dram_tensor("xqT", [E, S], BF16, kind="ExternalInput")
    xkT = nc.dram_tensor("xkT", [E, T], BF16, kind="ExternalInput")
    xvT = nc.dram_tensor("xvT", [E, T], BF16, kind="ExternalInput")
    wqT = nc.dram_tensor("wqT", [E, DG], BF16, kind="ExternalInput")
    wkT = nc.dram_tensor("wkT", [E, DG], BF16, kind="ExternalInput")
    wvT = nc.dram_tensor("wvT", [E, DG], BF16, kind="ExternalInput")
    wpT = nc.dram_tensor("wpT", [DG, E], BF16, kind="ExternalInput")
    bq = nc.dram_tensor("bq", [DG], F32, kind="ExternalInput")
    bk = nc.dram_tensor("bk", [DG], F32, kind="ExternalInput")
    bv = nc.dram_tensor("bv", [DG], F32, kind="ExternalInput")
    out = nc.dram_tensor("out", [S, E], F32, kind="ExternalOutput")

    with tile.TileContext(nc) as tc:
        with (
            tc.tile_pool(name="const", bufs=1) as cpool,
            tc.tile_pool(name="persist", bufs=1) as ppool,
            tc.tile_pool(name="wpool", bufs=1) as wpool,
            tc.tile_pool(name="spool", bufs=3) as spool,
            tc.tile_pool(name="epool", bufs=10) as epool,
            tc.tile_pool(name="npool", bufs=2) as npool,
            tc.tile_pool(name="n1pool", bufs=2) as n1pool,
            tc.tile_pool(name="ypool", bufs=2) as ypool,
            tc.tile_pool(name="opool", bufs=2) as opool,
            tc.tile_pool(name="stp", bufs=2, space="PSUM") as stp,
            tc.tile_pool(name="ytp", bufs=2, space="PSUM") as ytp,
            tc.tile_pool(name="opp", bufs=2, space="PSUM") as opp,
        ):
            # weights for K first so the K projection starts immediately
            wk_sb = wpool.tile([128, EB, DG], BF16, tag="wk", name="wk")
            wkr = wkT.rearrange("(eb p) d -> p eb d", p=128)
            nc.sync.dma_start(wk_sb[:, 0 : EB // 2, :], wkr[:, 0 : EB // 2, :])
            nc.gpsimd.dma_start(wk_sb[:, EB // 2 : EB, :], wkr[:, EB // 2 : EB, :])
            qt_sb = ppool.tile([128, DB, S], BF16, tag="qt")
            kt_sb = ppool.tile([128, DB, T], BF16, tag="kt")
            v_sb = ppool.tile([128, TB, NH * VW], BF16, tag="v")

            def proj_db(xs, w_sb, b_sb, o_sb, pc, db):
                ps = opp.tile([128, 512], F32, tag="op", name="psp")
                for kb in range(EB):
                    nc.tensor.matmul(
                        ps[:],
                        w_sb[:, kb, db * 128 : (db + 1) * 128],
                        xs[:, kb, :],
                        start=(kb == 0),
                        stop=(kb == EB - 1),
                    )
                nc.vector.tensor_scalar_add(
                    o_sb[:, db, pc * 512 : (pc + 1) * 512],
                    ps[:],
                    b_sb[:, db : db + 1],
                )

            def x_chunk(src, pc, tag, split=False):
                xs = spool.tile([128, EB, 512], BF16, tag=tag, name=tag)
                r = src.rearrange("(eb p) s -> p eb s", p=128)[
                    :, :, pc * 512 : (pc + 1) * 512
                ]
                if split:
                    # two DMAs so the first kb matmuls start sooner
                    nc.sync.dma_start(xs[:, 0 : EB // 2, :], r[:, 0 : EB // 2, :])
                    nc.sync.dma_start(xs[:, EB // 2 : EB, :], r[:, EB // 2 : EB, :])
                else:
                    nc.sync.dma_start(xs[:], r)
                return xs

            def proj_chunk(src, w_sb, b_sb, o_sb, pc, tag="xs", split=False,
                           dbs=None):
                xs = x_chunk(src, pc, tag, split=split)
                for db in dbs if dbs is not None else range(DB):
                    proj_db(xs, w_sb, b_sb, o_sb, pc, db)
                return xs

            # K chunk 0 (split x DMA for a faster first matmul); the
            # strided bias DMA (512 descriptors, ~6us) is emitted after the
            # x chunk so it does not block the critical first transfers.
            xk0 = x_chunk(xkT, 0, "xs", split=True)
            bk_sb = cpool.tile([128, DB], F32, tag="bk")
            nc.sync.dma_start(bk_sb[:], bk.rearrange("(db p) -> p db", p=128))
            for db in range(DB):
                proj_db(xk0, wk_sb, bk_sb, kt_sb, 0, db)

            wq_sb = wpool.tile([128, EB, DG], BF16, tag="wq", name="wq")
            nc.sync.dma_start(wq_sb[:], wqT.rearrange("(eb p) d -> p eb d", p=128))
            # Q chunk 0, head-pair 0 only (unblocks attention chunk 0);
            # the other head-pairs are woven into the first V group below.
            xq0 = x_chunk(xqT, 0, "xq", split=True)
            bq_sb = cpool.tile([128, DB], F32, tag="bq")
            nc.sync.dma_start(bq_sb[:], bq.rearrange("(db p) -> p db", p=128))
            proj_db(xq0, wq_sb, bq_sb, qt_sb, 0, 0)

            wv_sb = wpool.tile([128, EB, DG], BF16, tag="wv", name="wv")
            nc.sync.dma_start(wv_sb[:], wvT.rearrange("(eb p) d -> p eb d", p=128))

            filler = []

            def pump():
                if filler:
                    filler.pop(0)()

            def v_tile(xs, tb):
                t4 = tb % 4
                ps = opp.tile([128, DG], F32, tag="op", name="psv")
                for kb in range(EB):
                    nc.tensor.matmul(
                        ps[:],
                        xs[:, kb, t4 * 128 : (t4 + 1) * 128],
                        wv_sb[:, kb, :],
                        start=(kb == 0),
                        stop=(kb == EB - 1),
                    )
                nc.vector.tensor_copy(
                    v_sb[:, tb, :].rearrange("p (h w) -> p h w", w=VW)[:, :, 0:HD],
                    ps[:].rearrange("p (h w) -> p h w", w=HD),
                )
                nc.vector.memset(
                    v_sb[:, tb, :].rearrange("p (h w) -> p h w", w=VW)[:, :, HD:VW],
                    1.0,
                )

            def normalize(ye, hpair, yt_sb):
                for hi, h in ((0, hpair[0]), (1, hpair[1])):
                    db_, rh = h // 2, (h % 2) * 64
                    sp = n1pool.tile([128, SP], F32, tag="sp", name="sp")
                    nc.sync.dma_start(sp[:], ye[hi][64:65, :])
                    nc.vector.reciprocal(sp[:], sp[:])
                    rs = n1pool.tile([1, CH], F32, tag="rs", name="rs")
                    nc.sync.dma_start(rs[:], sp[:])
                    rbc = n1pool.tile([64, CH], F32, tag="rbc", name="rbc")
                    nc.gpsimd.partition_broadcast(rbc[:], rs[:])
                    if rh == 0:
                        dst = yt_sb[0:64, db_, :]
                        nc.vector.tensor_tensor(
                            dst, ye[hi][0:64, :], rbc[:], mybir.AluOpType.mult
                        )
                        nc.vector.tensor_scalar_add(dst, dst, bv_sb[:, h : h + 1])
                    else:
                        tmp = n1pool.tile([64, CH], BF16, tag="tmp", name="tmp")
                        nc.vector.tensor_tensor(
                            tmp[:], ye[hi][0:64, :], rbc[:], mybir.AluOpType.mult
                        )
                        nc.vector.tensor_scalar_add(
                            tmp[:], tmp[:], bv_sb[:, h : h + 1]
                        )
                        nc.sync.dma_start(yt_sb[64:128, db_, :], tmp[:])

            def outproj_unit(yt_sb, s0, sb, jc):
                def emit():
                    op = opp.tile([128, 512], F32, tag="op", name="op")
                    for ib in range(DB):
                        nc.tensor.matmul(
                            op[:],
                            yt_sb[:, ib, sb * 128 : (sb + 1) * 128],
                            wp_sb[:, ib, jc * 512 : (jc + 1) * 512],
                            start=(ib == 0),
                            stop=(ib == DB - 1),
                        )
                    ob = opool.tile([128, 512], F32, tag="ob")
                    nc.vector.tensor_copy(ob[:], op[:])
                    nc.sync.dma_start(
                        out[
                            s0 + sb * 128 : s0 + (sb + 1) * 128,
                            jc * 512 : (jc + 1) * 512,
                        ],
                        ob[:],
                    )

                return emit

            def qproj_unit(pc, db):
                def emit():
                    if db == 0:
                        qproj_unit.xs = x_chunk(xqT, pc, "xq")
                    proj_db(qproj_unit.xs, wq_sb, bq_sb, qt_sb, pc, db)

                return emit

            def attn_tb(hp, s0, yt0, yt1, tb):
                h0, h1 = 2 * hp, 2 * hp + 1
                st = stp.tile([128, 2 * CH], F32, tag="st", name="st")
                nc.tensor.matmul(
                    st[:, 0:CH],
                    kt_sb[0:64, hp, tb * 128 : (tb + 1) * 128],
                    qt_sb[0:64, hp, s0 : s0 + CH],
                    start=True,
                    stop=True,
                    tile_position=(0, 0),
                )
                nc.tensor.matmul(
                    st[:, CH : 2 * CH],
                    kt_sb[64:128, hp, tb * 128 : (tb + 1) * 128],
                    qt_sb[64:128, hp, s0 : s0 + CH],
                    start=True,
                    stop=True,
                    tile_position=(64, 0),
                )
                e_tb = epool.tile([128, 2 * CH], BF16, tag="e")
                nc.scalar.activation(e_tb[:], st[:], AF.Exp, scale=0.125)
                nc.tensor.matmul(
                    yt0[:],
                    v_sb[:, tb, h0 * VW : (h0 + 1) * VW],
                    e_tb[:, 0:CH],
                    start=(tb == 0),
                    stop=(tb == TB - 1),
                )
                nc.tensor.matmul(
                    yt1[:],
                    v_sb[:, tb, h1 * VW : (h1 + 1) * VW],
                    e_tb[:, CH : 2 * CH],
                    start=(tb == 0),
                    stop=(tb == TB - 1),
                )

            def attn_hp_tail(hp, yt_sb, yt0, yt1):
                h0, h1 = 2 * hp, 2 * hp + 1
                ye = [
                    npool.tile([65, CH], F32, tag="ye0", name="ye0"),
                    npool.tile([65, CH], F32, tag="ye1", name="ye1"),
                ]
                nc.vector.tensor_copy(ye[0][:], yt0[:])
                nc.vector.tensor_copy(ye[1][:], yt1[:])
                normalize(ye, (h0, h1), yt_sb)

            # ---- chunk 0, head-pair 0: interleaved with K chunks 1-3 and
            # V production so the exp stream starts as early as possible.
            yt_c0 = ypool.tile([128, DB, CH], BF16, tag="yt", name="yt_sb")
            yt0 = ytp.tile([65, CH], F32, tag="ytp", name="yt0")
            yt1 = ytp.tile([65, CH], F32, tag="ytp", name="yt1")
            for pcv in range(T // 512):
                xs = x_chunk(xvT, pcv, "xs", split=True)
                for t4 in range(4):
                    tb = pcv * 4 + t4
                    v_tile(xs, tb)
                    attn_tb(0, 0, yt0, yt1, tb)
                    if pcv == 0 and t4 < 3:
                        proj_db(xq0, wq_sb, bq_sb, qt_sb, 0, t4 + 1)
                if pcv + 1 < T // 512:
                    proj_chunk(xkT, wk_sb, bk_sb, kt_sb, pcv + 1)
            bv_sb = cpool.tile([HD, NH], F32, tag="bv")
            nc.sync.dma_start(bv_sb[:], bv.rearrange("(h p) -> p h", p=HD))
            wp_sb = cpool.tile([128, DB, E], BF16, tag="wp")
            nc.sync.dma_start(wp_sb[:], wpT.rearrange("(db p) e -> p db e", p=128))
            attn_hp_tail(0, yt_c0, yt0, yt1)

            # ---- remaining head-pairs / chunks, filler-pumped ----
            for c in range(NCH):
                s0 = c * CH
                if c == 0:
                    yt_sb = yt_c0
                else:
                    yt_sb = ypool.tile([128, DB, CH], BF16, tag="yt", name="yt_sb")
                for hp in range(DB):
                    if c == 0 and hp == 0:
                        continue
                    yt0 = ytp.tile([65, CH], F32, tag="ytp", name="yt0")
                    yt1 = ytp.tile([65, CH], F32, tag="ytp", name="yt1")
                    for tb in range(TB):
                        attn_tb(hp, s0, yt0, yt1, tb)
                        if tb % 5 == 2:
                            pump()
                    attn_hp_tail(hp, yt_sb, yt0, yt1)
                    if hp == 1 and c + 1 < NCH:
                        for pq in range(CH // 512):
                            pc = (c + 1) * (CH // 512) + pq
                            for db in range(DB):
                                filler.append(qproj_unit(pc, db))
                for sb in range(CH // 128):
                    for jc in range(E // 512):
                        filler.append(outproj_unit(yt_sb, s0, sb, jc))
            while filler:
                pump()

    nc.compile()
    return nc


_NC_CACHE = {}


def _get_nc(key, builder):
    if key not in _NC_CACHE:
        _NC_CACHE[key] = builder()
    return _NC_CACHE[key]


def kernel(query, key, value, Wq, bq, Wk, bk, Wv, bv, Wp, bp):
    query = np.asarray(query, np.float32)
    key = np.asarray(key, np.float32)
    value = np.asarray(value, np.float32)
    Wq, bq = np.asarray(Wq, np.float32), np.asarray(bq, np.float32)
    Wk, bk = np.asarray(Wk, np.float32), np.asarray(bk, np.float32)
    Wv, bv = np.asarray(Wv, np.float32), np.asarray(bv, np.float32)
    Wp, bp = np.asarray(Wp, np.float32), np.asarray(bp, np.float32)

    n, s, e = query.shape
    t = value.shape[1]
    assert (n, s, t, e) == (N_FULL, S_FULL, T_FULL, E_FULL)

    nc = _get_nc(
        "full",
        lambda: _build_nc(E_FULL, S_FULL, T_FULL, H_FULL // 2),
    )

    DG = (H_FULL // 2) * HD
    bf = ml_dtypes.bfloat16
    in_maps = []
    for c in range(N_CORES):
        b, g = c // 2, c % 2
        gs = slice(g * DG, (g + 1) * DG)
        in_maps.append(
            {
                "xqT": query[b].T.astype(bf),
                "xkT": key[b].T.astype(bf),
                "xvT": value[b].T.astype(bf),
                "wqT": Wq[gs, :].T.astype(bf),
                "wkT": Wk[gs, :].T.astype(bf),
                "wvT": Wv[gs, :].T.astype(bf),
                "wpT": Wp[:, gs].T.astype(bf),
                "bq": np.ascontiguousarray(bq[gs]),
                "bk": np.ascontiguousarray(bk[gs]),
                "bv": np.ascontiguousarray(bv[gs]),
            }
        )

    res = run_bass_kernel_spmd(
        nc, in_maps, list(range(N_CORES)), trace=TRACE, **TRACE_KW
    )
    LAST_RESULT[0] = res

    outp = np.empty((n, s, e), np.float32)
    for b in range(n):
        outp[b] = res.results[2 * b]["out"] + res.results[2 * b + 1]["out"] + bp
    return outp
